# revision 1
# baseline (speedup 1.0000x reference)
"""Trainium2 Bass kernel for DepthwiseXCorrAug.

Computes, for B=64 samples sharded 8-per-core across 8 NeuronCores:
  k = relu(bn(conv3x3_valid(kernel_in, w_k)))     # [B,256,5,5]
  s = relu(bn(conv3x3_same(search_in, w_s)))      # [B,256,31,31]
  out = per-sample per-channel xcorr(s, k), pad 2 # [B,256,31,31]

Device strategy (per core):
  - conv branches as float32r (TF32-like, full PE rate) matmuls over
    (ci-block x 3x3-tap) accumulated in PSUM; BN folded into weights on
    host, bias+ReLU applied by ScalarE on PSUM eviction.
  - depthwise xcorr as bf16 diagonal-weight matmuls: 16 concurrent 32x32
    PE tiles (4 channel-blocks x 4 samples) accumulate the 25 taps in PSUM.
"""

import sys
import types

sys.path.insert(0, "/opt/trn_rl_repo")

import numpy as np

import concourse.bass as bass
import concourse.mybir as mybir
import concourse.tile as tile
from concourse import bacc
from concourse.bass_utils import run_bass_kernel_spmd

EPS = 1e-5
N_CORES = 8
B, CIN, HID = 64, 256, 256
SPC = B // N_CORES  # samples per core

_cached_nc = None
last_results = None  # set by kernel(); used by test harness for profiling


def _round_fp32r(a: np.ndarray) -> np.ndarray:
    """Round fp32 to the PE's FP32R format (8-bit exp, 11-bit mantissa), RNE."""
    b = a.view(np.uint32).astype(np.uint64)
    lsb = (b >> 12) & 1
    r = b + 0x7FF + lsb
    return (r & ~np.uint64(0xFFF)).astype(np.uint32).view(np.float32)


def _build_program():
    f32 = mybir.dt.float32
    f32r = mybir.dt.float32r
    bf16 = mybir.dt.bfloat16
    RELU = mybir.ActivationFunctionType.Relu

    nc = bacc.Bacc("TRN2", target_bir_lowering=False, debug=False,
                   num_devices=N_CORES)

    wTs_d = [nc.dram_tensor(f"wTs{cb}", [128, 2304], f32r, kind="ExternalInput").ap()
             for cb in range(2)]
    wTk_d = [nc.dram_tensor(f"wTk{cb}", [128, 2304], f32r, kind="ExternalInput").ap()
             for cb in range(2)]
    xk_d = [nc.dram_tensor(f"xk{cb}", [128, 2304], f32r, kind="ExternalInput").ap()
            for cb in range(2)]
    xs_d = nc.dram_tensor("xs", [SPC, 2, 128, 33 * 34], f32r, kind="ExternalInput").ap()
    bk_d = nc.dram_tensor("bk", [2, 128, 1], f32, kind="ExternalInput").ap()
    bs_d = nc.dram_tensor("bs", [2, 128, 1], f32, kind="ExternalInput").ap()
    m32_d = nc.dram_tensor("m32", [128, 32], bf16, kind="ExternalInput").ap()
    out_d = nc.dram_tensor("out", [SPC, CIN, 31, 31], f32, kind="ExternalOutput").ap()
    out_flat = out_d.rearrange("s c h w -> s c (h w)")

    with tile.TileContext(nc) as tc:
        with tc.tile_pool(name="wp", bufs=1) as wp, \
             tc.tile_pool(name="spin", bufs=8) as spin_pool, \
             tc.tile_pool(name="spoutp", bufs=1) as spout_pool, \
             tc.tile_pool(name="stripp", bufs=1) as strip_pool, \
             tc.tile_pool(name="xop", bufs=8) as xout_pool, \
             tc.tile_pool(name="psc", bufs=4, space="PSUM") as psc, \
             tc.tile_pool(name="psx", bufs=4, space="PSUM") as psx_pool:

            # ---- persistent inputs ----
            wTs = [wp.tile([128, 2304], f32r, tag=f"wTs{cb}", name=f"wTs{cb}")
                   for cb in range(2)]
            wTk = [wp.tile([128, 2304], f32r, tag=f"wTk{cb}", name=f"wTk{cb}")
                   for cb in range(2)]
            xk = [wp.tile([128, 2304], f32r, tag=f"xk{cb}", name=f"xk{cb}")
                  for cb in range(2)]
            bk = [wp.tile([128, 1], f32, tag=f"bk{ob}", name=f"bk{ob}")
                  for ob in range(2)]
            bs = [wp.tile([128, 1], f32, tag=f"bs{ob}", name=f"bs{ob}")
                  for ob in range(2)]
            m32 = wp.tile([128, 32], bf16, tag="m32", name="m32")
            kf = [wp.tile([128, 200], f32, tag=f"kf{ob}", name=f"kf{ob}")
                  for ob in range(2)]

            # spin prefetch state (filled by prefetch_pair below)
            spin_views = {}

            def prefetch_pair(pair):
                s0 = pair * 2
                for s in (s0, s0 + 1):
                    for cb in range(2):
                        t_in = spin_pool.tile([128, 33 * 34], f32r,
                                              tag="spin", name=f"spin{s}_{cb}")
                        nc.sync.dma_start(t_in[:], xs_d[s, cb])
                        spin_views[(s, cb)] = t_in[:].rearrange(
                            "p (h w) -> p h w", h=33, w=34)

            for ob in range(2):
                nc.sync.dma_start(bk[ob][:], bk_d[ob])
                nc.sync.dma_start(bs[ob][:], bs_d[ob])
            nc.sync.dma_start(m32[:], m32_d)
            prefetch_pair(0)
            # big loads split into column chunks so no single DMA queue
            # serializes the critical path; conv_k inputs (wTk/xk) before the
            # pair-1 search tiles, which aren't needed until later
            CH = 576
            for cb in range(2):
                for c0 in range(0, 2304, CH):
                    nc.sync.dma_start(wTs[cb][:, c0:c0 + CH],
                                      wTs_d[cb][:, c0:c0 + CH])
            for cb in range(2):
                for c0 in range(0, 2304, CH):
                    nc.sync.dma_start(wTk[cb][:, c0:c0 + CH],
                                      wTk_d[cb][:, c0:c0 + CH])
                for c0 in range(0, 2304, CH):
                    nc.sync.dma_start(xk[cb][:, c0:c0 + CH],
                                      xk_d[cb][:, c0:c0 + CH])
            prefetch_pair(1)

            # ---- conv_k: all 8 samples batched on the free dim (N=256) ----
            def emit_conv_k():
                for ob in range(2):
                    pk = psc.tile([128, 512], f32, tag="conv", name=f"pk{ob}")
                    idx = 0
                    for cb in range(2):
                        for t in range(9):
                            nc.tensor.matmul(
                                pk[:, 0:256],
                                wTs_lhs(wTk, cb, t, ob),
                                xk[cb][:, t * 256:(t + 1) * 256],
                                start=(idx == 0), stop=(idx == 17))
                            idx += 1
                    nc.scalar.activation(kf[ob][:], pk[:, 0:200], RELU,
                                         bias=bk[ob][:, 0:1], scale=1.0)

            # ---- strips: bf16 diagonal weights for the xcorr ----
            strips = {}
            for s in range(SPC):
                for ob in range(2):
                    strips[(s, ob)] = strip_pool.tile(
                        [128, 800], bf16,
                        tag=f"strip{s}_{ob}", name=f"strip{s}_{ob}")

            def emit_strips():
                for ob in range(2):
                    for s in range(SPC):
                        st = strips[(s, ob)]
                        for t in range(25):
                            nc.vector.tensor_scalar(
                                st[:, t * 32:(t + 1) * 32], m32[:],
                                kf[ob][:, s * 25 + t:s * 25 + t + 1],
                                None, mybir.AluOpType.mult)

            # ---- spout tiles (bf16, zero borders) ----
            spout = {}
            for s in range(SPC):
                for ob in range(2):
                    sp = spout_pool.tile([128, 35 * 35], bf16,
                                         tag=f"spout{s}_{ob}", name=f"spout{s}_{ob}")
                    spout[(s, ob)] = sp
                    nc.gpsimd.memset(sp[:], 0.0)

            # ---- main: conv pairs interleaved with xcorr chunks ----
            def conv_s_pair(pair):
                s0 = pair * 2
                views = spin_views
                for ob in range(2):
                    ptiles = {}
                    for s in (s0, s0 + 1):
                        for ci, (y0, nr) in enumerate([(0, 16), (16, 15)]):
                            ptiles[(s, ci)] = psc.tile(
                                [128, 512], f32, tag="conv",
                                name=f"pc{s}_{ob}_{ci}")
                    idx = 0
                    for cb in range(2):
                        for t in range(9):
                            dy, dx = t // 3, t % 3
                            lhsT = wTs[cb][:, (t * 2 + ob) * 128:(t * 2 + ob + 1) * 128]
                            for s in (s0, s0 + 1):
                                for ci, (y0, nr) in enumerate([(0, 16), (16, 15)]):
                                    nc.tensor.matmul(
                                        ptiles[(s, ci)][:, 0:nr * 32],
                                        lhsT,
                                        views[(s, cb)][:, y0 + dy:y0 + dy + nr,
                                                       dx:dx + 32],
                                        start=(idx == 0), stop=(idx == 17))
                            idx += 1
                    for s in (s0, s0 + 1):
                        sov = spout[(s, ob)][:].rearrange(
                            "p (h w) -> p h w", h=35, w=35)
                        for ci, (y0, nr) in enumerate([(0, 16), (16, 15)]):
                            pv = ptiles[(s, ci)][:, 0:nr * 32].rearrange(
                                "p (h w) -> p h w", h=nr, w=32)
                            nc.scalar.activation(
                                sov[:, 2 + y0:2 + y0 + nr, 2:33],
                                pv[:, :, 0:31], RELU,
                                bias=bs[ob][:, 0:1], scale=1.0)

            def xcorr_chunk(g, ob, ci, pool=None, tag="xc"):
                    pool = pool or psx_pool
                    sovs = [spout[(g * 4 + j, ob)][:].rearrange(
                        "p (h w) -> p h w", h=35, w=35) for j in range(4)]
                    for (y0, nr) in [[(0, 16), (16, 15)][ci]]:
                        N = nr * 31
                        px = [pool.tile([128, 512], f32, tag=tag,
                                        name=f"px{g}_{ob}_{y0}_{i}")
                              for i in range(4)]
                        for t in range(25):
                            dy, dx = t // 5, t % 5
                            for i in range(4):
                                for j in range(4):
                                    st = strips[(g * 4 + j, ob)]
                                    nc.tensor.matmul(
                                        px[i][32 * j:32 * j + 32, 0:N],
                                        st[32 * i:32 * i + 32, t * 32:(t + 1) * 32],
                                        sovs[j][32 * i:32 * i + 32,
                                                y0 + dy:y0 + dy + nr, dx:dx + 31],
                                        start=(t == 0), stop=(t == 24),
                                        tile_position=(32 * i, 32 * j))
                        for i in range(4):
                            xo = xout_pool.tile([128, 496], f32, tag="xo",
                                                name=f"xo{g}_{ob}_{y0}_{i}")
                            if i % 2 == 0:
                                nc.vector.tensor_copy(xo[:, 0:N], px[i][:, 0:N])
                            else:
                                nc.scalar.copy(xo[:, 0:N], px[i][:, 0:N])
                            dst = out_flat[g * 4:g * 4 + 4,
                                           ob * 128 + 32 * i:ob * 128 + 32 * i + 32,
                                           y0 * 31:y0 * 31 + N]
                            nc.sync.dma_start(dst, xo[:, 0:N])

            conv_s_pair(0)
            prefetch_pair(2)
            emit_conv_k()
            emit_strips()
            conv_s_pair(1)
            prefetch_pair(3)
            xcorr_chunk(0, 0, 0)
            conv_s_pair(2)
            xcorr_chunk(0, 0, 1)
            conv_s_pair(3)
            for n, args in enumerate([(0, 1, 0), (0, 1, 1), (1, 0, 0),
                                      (1, 0, 1), (1, 1, 0), (1, 1, 1)]):
                if n % 2 == 0:
                    xcorr_chunk(*args)
                else:
                    xcorr_chunk(*args, pool=psc, tag="conv")

    nc.compile()
    return nc


def wTs_lhs(w, cb, t, ob):
    return w[cb][:, (t * 2 + ob) * 128:(t * 2 + ob + 1) * 128]


def _host_prep(kernel, search, w_k, g_k, b_k, m_k, v_k, w_s, g_s, b_s, m_s, v_s):
    import ml_dtypes

    def fold(w, g, b, m, v):
        scale = g / np.sqrt(v + EPS)
        return (w * scale[:, None, None, None]).astype(np.float32), \
               (b - m * scale).astype(np.float32)

    wkf, bias_k = fold(w_k, g_k, b_k, m_k, v_k)
    wsf, bias_s = fold(w_s, g_s, b_s, m_s, v_s)

    def packT(w):  # [o, ci, 3, 3] -> [cb, ci, (t, ob, o)] fp32r
        arr = w.reshape(2, 128, 2, 128, 9).transpose(2, 3, 4, 0, 1)
        return _round_fp32r(np.ascontiguousarray(arr, dtype=np.float32)
                            ).reshape(2, 128, 2304)

    wTk = packT(wkf)
    wTs = packT(wsf)

    M32 = np.zeros((128, 32), dtype=np.float32)
    for p in range(128):
        M32[p, p % 32] = 1.0
    M32 = M32.astype(ml_dtypes.bfloat16)

    bk = np.ascontiguousarray(bias_k.reshape(2, 128, 1))
    bs = np.ascontiguousarray(bias_s.reshape(2, 128, 1))

    in_maps = []
    for core in range(N_CORES):
        kin = kernel[core * SPC:(core + 1) * SPC]
        sin = search[core * SPC:(core + 1) * SPC]

        Xk = np.zeros((2, 128, 9, 256), dtype=np.float32)
        for t in range(9):
            dy, dx = t // 3, t % 3
            p = kin[:, :, dy:dy + 5, dx:dx + 5].reshape(SPC, 2, 128, 25)
            Xk[:, :, t, :200] = p.transpose(1, 2, 0, 3).reshape(2, 128, 200)
        Xk = _round_fp32r(Xk).reshape(2, 128, 2304)

        Xs = np.zeros((SPC, 2, 128, 33, 34), dtype=np.float32)
        Xs[:, :, :, 1:32, 1:32] = sin.reshape(SPC, 2, 128, 31, 31)
        Xs = _round_fp32r(Xs).reshape(SPC, 2, 128, 33 * 34)

        in_maps.append({
            "wTs0": wTs[0], "wTs1": wTs[1],
            "wTk0": wTk[0], "wTk1": wTk[1],
            "xk0": Xk[0], "xk1": Xk[1],
            "xs": Xs, "bk": bk, "bs": bs, "m32": M32,
        })
    return in_maps


def kernel(kernel, search, w_k, g_k, b_k, m_k, v_k, w_s, g_s, b_s, m_s, v_s,
           _trace=False):
    global _cached_nc, last_results
    args = [np.ascontiguousarray(np.asarray(x, dtype=np.float32)) for x in
            (kernel, search, w_k, g_k, b_k, m_k, v_k, w_s, g_s, b_s, m_s, v_s)]
    if _cached_nc is None:
        _cached_nc = _build_program()
    nc = _cached_nc
    in_maps = _host_prep(*args)
    res = run_bass_kernel_spmd(nc, in_maps, core_ids=list(range(N_CORES)),
                               trace=_trace)
    last_results = res
    out = np.concatenate([res.results[i]["out"] for i in range(N_CORES)], axis=0)
    return np.ascontiguousarray(out.astype(np.float32))



# revision 10
# speedup vs baseline: 1.1103x; 1.1103x over previous
"""Trainium2 Bass kernel for DepthwiseXCorrAug.

Computes, for B=64 samples sharded 8-per-core across 8 NeuronCores:
  k = relu(bn(conv3x3_valid(kernel_in, w_k)))     # [B,256,5,5]
  s = relu(bn(conv3x3_same(search_in, w_s)))      # [B,256,31,31]
  out = per-sample per-channel xcorr(s, k), pad 2 # [B,256,31,31]

Device strategy (per core):
  - everything in bf16 on the PE (weights, activations); accumulate f32 PSUM.
  - conv branches as (ci-block x 3x3-tap) matmuls accumulated in PSUM; BN
    folded into weights on host, bias+ReLU applied by ScalarE on eviction.
  - depthwise xcorr as bf16 diagonal-weight matmuls: 16 concurrent 32x32
    PE tiles (4 channel-blocks x 4 samples) accumulate the 25 taps in PSUM.
  - conv_k first (small deps -> PE starts early), then conv_s pairs, then
    xcorr chunks; outputs stream out as bf16, host converts to f32.
"""

import sys

sys.path.insert(0, "/opt/trn_rl_repo")

import numpy as np

import concourse.bass as bass
import concourse.mybir as mybir
import concourse.tile as tile
from concourse import bacc
from concourse.bass_utils import run_bass_kernel_spmd

EPS = 1e-5
N_CORES = 8
B, CIN, HID = 64, 256, 256
SPC = B // N_CORES  # samples per core

_cached_nc = None
last_results = None  # set by kernel(); used by test harness for profiling


def _build_program():
    f32 = mybir.dt.float32
    bf16 = mybir.dt.bfloat16
    RELU = mybir.ActivationFunctionType.Relu

    nc = bacc.Bacc("TRN2", target_bir_lowering=False, debug=False,
                   num_devices=N_CORES)

    wTs_d = [nc.dram_tensor(f"wTs{cb}", [128, 2304], bf16, kind="ExternalInput").ap()
             for cb in range(2)]
    wTk_d = [nc.dram_tensor(f"wTk{cb}", [128, 2304], bf16, kind="ExternalInput").ap()
             for cb in range(2)]
    xk_d = [nc.dram_tensor(f"xk{cb}", [128, 1800], bf16, kind="ExternalInput").ap()
            for cb in range(2)]
    xs_d = nc.dram_tensor("xs", [SPC, 2, 128, 33 * 34], bf16, kind="ExternalInput").ap()
    bk_d = nc.dram_tensor("bk", [2, 128, 1], f32, kind="ExternalInput").ap()
    bs_d = nc.dram_tensor("bs", [2, 128, 1], f32, kind="ExternalInput").ap()
    m32rep_d = nc.dram_tensor("m32rep", [128, 800], bf16, kind="ExternalInput").ap()
    out_d = nc.dram_tensor("out", [SPC, CIN, 31, 31], bf16, kind="ExternalOutput").ap()
    out_flat = out_d.rearrange("s c h w -> s c (h w)")

    with tile.TileContext(nc) as tc:
        with tc.tile_pool(name="wp", bufs=1) as wp, \
             tc.tile_pool(name="spin", bufs=16) as spin_pool, \
             tc.tile_pool(name="spoutp", bufs=1) as spout_pool, \
             tc.tile_pool(name="stripp", bufs=1) as strip_pool, \
             tc.tile_pool(name="xop", bufs=8) as xout_pool, \
             tc.tile_pool(name="ps", bufs=8, space="PSUM") as ps:

            # ---- persistent inputs ----
            wTs = [wp.tile([128, 2304], bf16, tag=f"wTs{cb}", name=f"wTs{cb}")
                   for cb in range(2)]
            wTk = [wp.tile([128, 2304], bf16, tag=f"wTk{cb}", name=f"wTk{cb}")
                   for cb in range(2)]
            xk = [wp.tile([128, 1800], bf16, tag=f"xk{cb}", name=f"xk{cb}")
                  for cb in range(2)]
            bk = [wp.tile([128, 1], f32, tag=f"bk{ob}", name=f"bk{ob}")
                  for ob in range(2)]
            bs = [wp.tile([128, 1], f32, tag=f"bs{ob}", name=f"bs{ob}")
                  for ob in range(2)]
            m32rep = wp.tile([128, 800], bf16, tag="m32rep", name="m32rep")
            kf = [wp.tile([128, 200], f32, tag=f"kf{ob}", name=f"kf{ob}")
                  for ob in range(2)]

            # ---- spout tiles (bf16); zero only the 2-wide borders ----
            spout = {}
            for s in range(SPC):
                for ob in range(2):
                    sp = spout_pool.tile([128, 35 * 35], bf16,
                                         tag=f"spout{s}_{ob}", name=f"spout{s}_{ob}")
                    spout[(s, ob)] = sp
                    eng = nc.vector if (s + ob) % 2 == 0 else nc.gpsimd
                    # zero the 2-wide border frame; interior is written by
                    # the conv_s activation
                    v = sp[:].rearrange("p (r c) -> p r c", r=35, c=35)
                    eng.memset(sp[:, 0:70], 0.0)
                    eng.memset(sp[:, 1155:1225], 0.0)
                    eng.memset(v[:, 2:33, 0:2], 0.0)
                    eng.memset(v[:, 2:33, 33:35], 0.0)

            # spin prefetch state
            spin_views = {}

            def prefetch_pair(pair):
                s0 = pair * 2
                for s in (s0, s0 + 1):
                    for cb in range(2):
                        t_in = spin_pool.tile([128, 33 * 34], bf16,
                                              tag="spin", name=f"spin{s}_{cb}")
                        nc.sync.dma_start(t_in[:], xs_d[s, cb])
                        spin_views[(s, cb)] = t_in[:].rearrange(
                            "p (h w) -> p h w", h=33, w=34)

            # ---- DMA order: conv_k deps first so PE starts ASAP ----
            for ob in range(2):
                nc.sync.dma_start(bk[ob][:], bk_d[ob])
                nc.sync.dma_start(bs[ob][:], bs_d[ob])
            nc.sync.dma_start(m32rep[:], m32rep_d)
            CH = 1152
            for cb in range(2):
                for c0 in range(0, 2304, CH):
                    nc.sync.dma_start(wTk[cb][:, c0:c0 + CH],
                                      wTk_d[cb][:, c0:c0 + CH])
                for c0 in range(0, 1800, 900):
                    nc.sync.dma_start(xk[cb][:, c0:c0 + 900],
                                      xk_d[cb][:, c0:c0 + 900])
            prefetch_pair(0)
            for cb in range(2):
                for c0 in range(0, 2304, CH):
                    nc.sync.dma_start(wTs[cb][:, c0:c0 + CH],
                                      wTs_d[cb][:, c0:c0 + CH])
            for p in (1, 2, 3):
                prefetch_pair(p)

            # ---- conv_k: all 8 samples batched on the free dim (N=256) ----
            def emit_conv_k():
                for ob in range(2):
                    pk = ps.tile([128, 512], f32, tag="mm", name=f"pk{ob}")
                    idx = 0
                    for cb in range(2):
                        for t in range(9):
                            nc.tensor.matmul(
                                pk[:, 0:200],
                                wTk[cb][:, (t * 2 + ob) * 128:(t * 2 + ob + 1) * 128],
                                xk[cb][:, t * 200:(t + 1) * 200],
                                start=(idx == 0), stop=(idx == 17))
                            idx += 1
                    nc.scalar.activation(kf[ob][:], pk[:, 0:200], RELU,
                                         bias=bk[ob][:, 0:1], scale=1.0)

            # ---- strips: bf16 diagonal weights, one DVE op per (s, ob) ----
            strips = {}
            for s in range(SPC):
                for ob in range(2):
                    strips[(s, ob)] = strip_pool.tile(
                        [128, 800], bf16,
                        tag=f"strip{s}_{ob}", name=f"strip{s}_{ob}")

            def emit_strips():
                for ob in range(2):
                    for s in range(SPC):
                        kfb = kf[ob][:, s * 25:(s + 1) * 25].unsqueeze(
                            -1).broadcast_to([128, 25, 32])
                        nc.vector.tensor_tensor(
                            strips[(s, ob)][:], m32rep[:], kfb,
                            mybir.AluOpType.mult)

            # ---- conv_s: one pair of samples, both ob blocks ----
            def conv_s_pair(pair):
                s0 = pair * 2
                views = spin_views
                for ob in range(2):
                    ptiles = {}
                    for s in (s0, s0 + 1):
                        for ci in range(2):
                            ptiles[(s, ci)] = ps.tile(
                                [128, 512], f32, tag="mm",
                                name=f"pc{s}_{ob}_{ci}")
                    idx = 0
                    for cb in range(2):
                        for t in range(9):
                            dy, dx = t // 3, t % 3
                            lhsT = wTs[cb][:, (t * 2 + ob) * 128:(t * 2 + ob + 1) * 128]
                            for s in (s0, s0 + 1):
                                for ci, (y0, nr) in enumerate([(0, 16), (16, 15)]):
                                    nc.tensor.matmul(
                                        ptiles[(s, ci)][:, 0:nr * 31],
                                        lhsT,
                                        views[(s, cb)][:, y0 + dy:y0 + dy + nr,
                                                       dx:dx + 31],
                                        start=(idx == 0), stop=(idx == 17))
                            idx += 1
                    for s in (s0, s0 + 1):
                        sov = spout[(s, ob)][:].rearrange(
                            "p (h w) -> p h w", h=35, w=35)
                        for ci, (y0, nr) in enumerate([(0, 16), (16, 15)]):
                            pv = ptiles[(s, ci)][:, 0:nr * 31].rearrange(
                                "p (h w) -> p h w", h=nr, w=31)
                            nc.scalar.activation(
                                sov[:, 2 + y0:2 + y0 + nr, 2:33],
                                pv[:, :, :], RELU,
                                bias=bs[ob][:, 0:1], scale=1.0)

            # ---- xcorr: merged-ci chunk per (g, ob): 8 PSUM banks ----
            CI_SPEC = [(0, 16), (16, 15)]

            def xcorr_chunk(g, ob):
                sovs = [spout[(g * 4 + j, ob)][:].rearrange(
                    "p (h w) -> p h w", h=35, w=35) for j in range(4)]
                px = {}
                for ci in range(2):
                    for i in range(4):
                        px[(ci, i)] = ps.tile([128, 512], f32, tag="mm",
                                              name=f"px{g}_{ob}_{ci}_{i}")
                for t in range(25):
                    dy, dx = t // 5, t % 5
                    for j in range(4):
                        for i in range(4):
                            st = strips[(g * 4 + j, ob)]
                            lhsT = st[32 * i:32 * i + 32, t * 32:(t + 1) * 32]
                            nc.tensor.ldweights(
                                lhsT, tile_position=(32 * i, 32 * j))
                            for ci, (y0, nr) in enumerate(CI_SPEC):
                                nc.tensor.matmul(
                                    px[(ci, i)][32 * j:32 * j + 32, 0:nr * 31],
                                    lhsT,
                                    sovs[j][32 * i:32 * i + 32,
                                            y0 + dy:y0 + dy + nr, dx:dx + 31],
                                    start=(t == 0), stop=(t == 24),
                                    tile_position=(32 * i, 32 * j))
                n_ev = 0
                for ci, (y0, nr) in enumerate(CI_SPEC):
                    N = nr * 31
                    for i in range(4):
                        xo = xout_pool.tile([128, 496], bf16, tag="xo",
                                            name=f"xo{g}_{ob}_{ci}_{i}")
                        if n_ev % 2 == 0:
                            nc.vector.tensor_copy(xo[:, 0:N], px[(ci, i)][:, 0:N])
                        else:
                            nc.scalar.copy(xo[:, 0:N], px[(ci, i)][:, 0:N])
                        n_ev += 1
                        dst = out_flat[g * 4:g * 4 + 4,
                                       ob * 128 + 32 * i:ob * 128 + 32 * i + 32,
                                       y0 * 31:y0 * 31 + N]
                        nc.sync.dma_start(dst, xo[:, 0:N])

            emit_conv_k()
            emit_strips()
            conv_s_pair(0)
            conv_s_pair(1)
            xcorr_chunk(0, 0)
            xcorr_chunk(0, 1)
            conv_s_pair(2)
            conv_s_pair(3)
            xcorr_chunk(1, 0)
            xcorr_chunk(1, 1)

    nc.compile()
    return nc


def _host_prep(kernel, search, w_k, g_k, b_k, m_k, v_k, w_s, g_s, b_s, m_s, v_s):
    import ml_dtypes
    bf16 = ml_dtypes.bfloat16

    def fold(w, g, b, m, v):
        scale = g / np.sqrt(v + EPS)
        return (w * scale[:, None, None, None]).astype(np.float32), \
               (b - m * scale).astype(np.float32)

    wkf, bias_k = fold(w_k, g_k, b_k, m_k, v_k)
    wsf, bias_s = fold(w_s, g_s, b_s, m_s, v_s)

    def packT(w):  # [o, ci, 3, 3] -> [cb, ci, (t, ob, o)] bf16
        arr = w.reshape(2, 128, 2, 128, 9).transpose(2, 3, 4, 0, 1)
        return np.ascontiguousarray(arr, dtype=np.float32).astype(
            bf16).reshape(2, 128, 2304)

    wTk = packT(wkf)
    wTs = packT(wsf)

    M32 = np.zeros((128, 32), dtype=np.float32)
    for p in range(128):
        M32[p, p % 32] = 1.0
    M32REP = np.tile(M32, (1, 25)).astype(bf16)

    bk = np.ascontiguousarray(bias_k.reshape(2, 128, 1))
    bs = np.ascontiguousarray(bias_s.reshape(2, 128, 1))

    in_maps = []
    for core in range(N_CORES):
        kin = kernel[core * SPC:(core + 1) * SPC]
        sin = search[core * SPC:(core + 1) * SPC]

        Xk = np.zeros((2, 128, 9, 200), dtype=np.float32)
        for t in range(9):
            dy, dx = t // 3, t % 3
            p = kin[:, :, dy:dy + 5, dx:dx + 5].reshape(SPC, 2, 128, 25)
            Xk[:, :, t, :] = p.transpose(1, 2, 0, 3).reshape(2, 128, 200)
        Xk = Xk.astype(bf16).reshape(2, 128, 1800)

        Xs = np.zeros((SPC, 2, 128, 33, 34), dtype=np.float32)
        Xs[:, :, :, 1:32, 1:32] = sin.reshape(SPC, 2, 128, 31, 31)
        Xs = Xs.astype(bf16).reshape(SPC, 2, 128, 33 * 34)

        in_maps.append({
            "wTs0": wTs[0], "wTs1": wTs[1],
            "wTk0": wTk[0], "wTk1": wTk[1],
            "xk0": Xk[0], "xk1": Xk[1],
            "xs": Xs, "bk": bk, "bs": bs, "m32rep": M32REP,
        })
    return in_maps


def kernel(kernel, search, w_k, g_k, b_k, m_k, v_k, w_s, g_s, b_s, m_s, v_s,
           _trace=False):
    global _cached_nc, last_results
    args = [np.ascontiguousarray(np.asarray(x, dtype=np.float32)) for x in
            (kernel, search, w_k, g_k, b_k, m_k, v_k, w_s, g_s, b_s, m_s, v_s)]
    if _cached_nc is None:
        _cached_nc = _build_program()
    nc = _cached_nc
    in_maps = _host_prep(*args)
    res = run_bass_kernel_spmd(nc, in_maps, core_ids=list(range(N_CORES)),
                               trace=_trace)
    last_results = res
    out = np.concatenate([res.results[i]["out"] for i in range(N_CORES)], axis=0)
    return np.ascontiguousarray(out.astype(np.float32))


# revision 27
# speedup vs baseline: 1.4212x; 1.2801x over previous
"""Trainium2 Bass kernel for DepthwiseXCorrAug.

Computes, for B=64 samples sharded 8-per-core across 8 NeuronCores:
  k = relu(bn(conv3x3_valid(kernel_in, w_k)))     # [B,256,5,5]
  s = relu(bn(conv3x3_same(search_in, w_s)))      # [B,256,31,31]
  out = per-sample per-channel xcorr(s, k), pad 2 # [B,256,31,31]

Device strategy (per core):
  - everything in bf16 on the PE (weights, activations); accumulate f32 PSUM.
  - conv branches as (ci-block x 3x3-tap) matmuls accumulated in PSUM; BN
    folded into weights on host, bias+ReLU applied by ScalarE on eviction.
  - depthwise xcorr as bf16 diagonal-weight matmuls: 16 concurrent 32x32
    PE tiles (4 channel-blocks x 4 samples) accumulate the 25 taps in PSUM.
  - conv_k first (small deps -> PE starts early), then conv_s pairs, then
    xcorr chunks; outputs stream out as bf16, host converts to f32.
"""

import sys

sys.path.insert(0, "/opt/trn_rl_repo")

import numpy as np

import concourse.bass as bass
import concourse.mybir as mybir
import concourse.tile as tile
from concourse import bacc
from concourse.bass_utils import run_bass_kernel_spmd

EPS = 1e-5
N_CORES = 8
B, CIN, HID = 64, 256, 256
SPC = B // N_CORES  # samples per core

_cached_nc = None
last_results = None  # set by kernel(); used by test harness for profiling


def _build_program():
    f32 = mybir.dt.float32
    bf16 = mybir.dt.bfloat16
    RELU = mybir.ActivationFunctionType.Relu

    nc = bacc.Bacc("TRN2", target_bir_lowering=False, debug=False,
                   num_devices=N_CORES)

    wTs_d = [nc.dram_tensor(f"wTs{cb}", [128, 2304], bf16, kind="ExternalInput").ap()
             for cb in range(2)]
    wTk_d = [nc.dram_tensor(f"wTk{cb}", [128, 2304], bf16, kind="ExternalInput").ap()
             for cb in range(2)]
    xk_d = [nc.dram_tensor(f"xk{cb}", [128, 1800], bf16, kind="ExternalInput").ap()
            for cb in range(2)]
    xs_d = nc.dram_tensor("xs", [SPC, 128, 2 * 33 * 34], bf16, kind="ExternalInput").ap()
    bk_d = nc.dram_tensor("bk", [2, 128, 1], f32, kind="ExternalInput").ap()
    bs_d = nc.dram_tensor("bs", [2, 128, 1], f32, kind="ExternalInput").ap()
    m64rep_d = nc.dram_tensor("m64rep", [128, 1600], bf16, kind="ExternalInput").ap()
    out_d = nc.dram_tensor("out", [SPC, CIN, 31, 31], bf16, kind="ExternalOutput").ap()
    out_flat = out_d.rearrange("s c h w -> s c (h w)")

    with tile.TileContext(nc) as tc:
        with tc.tile_pool(name="wp", bufs=1) as wp, \
             tc.tile_pool(name="spin", bufs=8) as spin_pool, \
             tc.tile_pool(name="spoutp", bufs=1) as spout_pool, \
             tc.tile_pool(name="stripp", bufs=1) as strip_pool, \
             tc.tile_pool(name="xop", bufs=8) as xout_pool, \
             tc.tile_pool(name="ps", bufs=8, space="PSUM") as ps:

            # ---- persistent inputs ----
            wTs = [wp.tile([128, 2304], bf16, tag=f"wTs{cb}", name=f"wTs{cb}")
                   for cb in range(2)]
            wTk = [wp.tile([128, 2304], bf16, tag=f"wTk{cb}", name=f"wTk{cb}")
                   for cb in range(2)]
            xk = [wp.tile([128, 1800], bf16, tag=f"xk{cb}", name=f"xk{cb}")
                  for cb in range(2)]
            bk = [wp.tile([128, 1], f32, tag=f"bk{ob}", name=f"bk{ob}")
                  for ob in range(2)]
            bs = [wp.tile([128, 1], f32, tag=f"bs{ob}", name=f"bs{ob}")
                  for ob in range(2)]
            m64rep = wp.tile([128, 1600], bf16, tag="m64rep", name="m64rep")
            kf = [wp.tile([128, 200], f32, tag=f"kf{ob}", name=f"kf{ob}")
                  for ob in range(2)]

            # ---- spout tiles (bf16); zero only the 2-wide borders ----
            # 8 physical tiles, reused by samples s and s+4 (deps tracked)
            spout = {}
            for s in range(4):
                for ob in range(2):
                    sp = spout_pool.tile([128, 35 * 35], bf16,
                                         tag=f"spout{s}_{ob}", name=f"spout{s}_{ob}")
                    spout[(s, ob)] = sp
                    spout[(s + 4, ob)] = sp
                    eng = nc.vector if (s + ob) % 2 == 0 else nc.gpsimd
                    # zero the 2-wide border frame; interior is written by
                    # the conv_s activation
                    v = sp[:].rearrange("p (r c) -> p r c", r=35, c=35)
                    eng.memset(sp[:, 0:70], 0.0)
                    eng.memset(sp[:, 1155:1225], 0.0)
                    eng.memset(v[:, 2:33, 0:2], 0.0)
                    eng.memset(v[:, 2:33, 33:35], 0.0)

            # spin prefetch state
            spin_views = {}

            def prefetch_pair(pair):
                s0 = pair * 2
                for s in (s0, s0 + 1):
                    t_in = spin_pool.tile([128, 2 * 33 * 34], bf16,
                                          tag="spin", name=f"spin{s}")
                    nc.sync.dma_start(t_in[:], xs_d[s])
                    for cb in range(2):
                        spin_views[(s, cb)] = t_in[
                            :, cb * 1122:(cb + 1) * 1122].rearrange(
                            "p (h w) -> p h w", h=33, w=34)

            # ---- DMA order: conv_k deps first so PE starts ASAP ----
            for ob in range(2):
                nc.sync.dma_start(bk[ob][:], bk_d[ob])
                nc.sync.dma_start(bs[ob][:], bs_d[ob])
            nc.sync.dma_start(m64rep[:], m64rep_d)
            for cb in range(2):
                nc.sync.dma_start(wTk[cb][:], wTk_d[cb])
                nc.sync.dma_start(xk[cb][:], xk_d[cb])
            prefetch_pair(0)
            for cb in range(2):
                nc.sync.dma_start(wTs[cb][:], wTs_d[cb])
            for p in (1, 2, 3):
                prefetch_pair(p)

            # ---- conv_k: all 8 samples batched on the free dim (N=256) ----
            def emit_conv_k():
                for ob in range(2):
                    pk = ps.tile([128, 512], f32, tag="mm", name=f"pk{ob}")
                    idx = 0
                    for cb in range(2):
                        for t in range(9):
                            nc.tensor.matmul(
                                pk[:, 0:200],
                                wTk[cb][:, (t * 2 + ob) * 128:(t * 2 + ob + 1) * 128],
                                xk[cb][:, t * 200:(t + 1) * 200],
                                start=(idx == 0), stop=(idx == 17))
                            idx += 1
                    nc.scalar.activation(kf[ob][:], pk[:, 0:200], RELU,
                                         bias=bk[ob][:, 0:1], scale=1.0)

            # ---- strips: bf16 64-diag weights, one DVE op per (s, ob) ----
            # 8 physical tiles, reused by samples s and s+4
            strips = {}
            for s in range(4):
                for ob in range(2):
                    st = strip_pool.tile(
                        [128, 1600], bf16,
                        tag=f"strip{s}_{ob}", name=f"strip{s}_{ob}")
                    strips[(s, ob)] = st
                    strips[(s + 4, ob)] = st

            def emit_strips(samples):
                for ob in range(2):
                    for s in samples:
                        kfb = kf[ob][:, s * 25:(s + 1) * 25].unsqueeze(
                            -1).broadcast_to([128, 25, 64])
                        nc.vector.tensor_tensor(
                            strips[(s, ob)][:], m64rep[:], kfb,
                            mybir.AluOpType.mult)

            # ---- conv_s: one pair of samples, both ob blocks ----
            def conv_s_pair(pair):
                s0 = pair * 2
                views = spin_views
                for ob in range(2):
                    ptiles = {}
                    for s in (s0, s0 + 1):
                        for ci in range(2):
                            ptiles[(s, ci)] = ps.tile(
                                [128, 512], f32, tag="mm",
                                name=f"pc{s}_{ob}_{ci}")
                    idx = 0
                    for cb in range(2):
                        for t in range(9):
                            dy, dx = t // 3, t % 3
                            lhsT = wTs[cb][:, (t * 2 + ob) * 128:(t * 2 + ob + 1) * 128]
                            for s in (s0, s0 + 1):
                                for ci, (y0, nr) in enumerate([(0, 16), (16, 15)]):
                                    nc.tensor.matmul(
                                        ptiles[(s, ci)][:, 0:nr * 31],
                                        lhsT,
                                        views[(s, cb)][:, y0 + dy:y0 + dy + nr,
                                                       dx:dx + 31],
                                        start=(idx == 0), stop=(idx == 17))
                            idx += 1
                    for s in (s0, s0 + 1):
                        sov = spout[(s, ob)][:].rearrange(
                            "p (h w) -> p h w", h=35, w=35)
                        for ci, (y0, nr) in enumerate([(0, 16), (16, 15)]):
                            pv = ptiles[(s, ci)][:, 0:nr * 31].rearrange(
                                "p (h w) -> p h w", h=nr, w=31)
                            nc.scalar.activation(
                                sov[:, 2 + y0:2 + y0 + nr, 2:33],
                                pv[:, :, :], RELU,
                                bias=bs[ob][:, 0:1], scale=1.0)

            # ---- xcorr: 64x64-tile chunk per (q, ob): sample pair (2q, 2q+1)
            CI_SPEC = [(0, 16), (16, 15)]

            def xcorr_chunk(q, ob):
                sovs = [spout[(q * 2 + j, ob)][:].rearrange(
                    "p (h w) -> p h w", h=35, w=35) for j in range(2)]
                px = {}
                for ci in range(2):
                    for i in range(2):
                        px[(ci, i)] = ps.tile([128, 512], f32, tag="mm",
                                              name=f"px{q}_{ob}_{ci}_{i}")
                for t in range(25):
                    dy, dx = t // 5, t % 5
                    for i in range(2):
                        for j in range(2):
                            st = strips[(q * 2 + j, ob)]
                            lhsT = st[64 * i:64 * i + 64, t * 64:(t + 1) * 64]
                            for ci, (y0, nr) in enumerate(CI_SPEC):
                                nc.tensor.matmul(
                                    px[(ci, i)][64 * j:64 * j + 64, 0:nr * 31],
                                    lhsT,
                                    sovs[j][64 * i:64 * i + 64,
                                            y0 + dy:y0 + dy + nr, dx:dx + 31],
                                    start=(t == 0), stop=(t == 24),
                                    tile_position=(64 * i, 64 * j))
                n_ev = 0
                for ci, (y0, nr) in enumerate(CI_SPEC):
                    N = nr * 31
                    xo = xout_pool.tile([128, 1024], bf16, tag="xo",
                                        name=f"xo{q}_{ob}_{ci}")
                    for i in range(2):
                        if n_ev % 2 == 0:
                            nc.vector.tensor_copy(
                                xo[:, i * 512:i * 512 + N], px[(ci, i)][:, 0:N])
                        else:
                            nc.scalar.copy(
                                xo[:, i * 512:i * 512 + N], px[(ci, i)][:, 0:N])
                        n_ev += 1
                    for i in range(2):
                        dst = out_flat[q * 2:q * 2 + 2,
                                       ob * 128 + 64 * i:ob * 128 + 64 * i + 64,
                                       y0 * 31:y0 * 31 + N]
                        nc.sync.dma_start(dst, xo[:, i * 512:i * 512 + N])

            emit_conv_k()
            emit_strips(range(4))
            conv_s_pair(0)
            conv_s_pair(1)
            for q in (0, 1):
                xcorr_chunk(q, 0)
                xcorr_chunk(q, 1)
            emit_strips(range(4, SPC))
            conv_s_pair(2)
            conv_s_pair(3)
            for q in (2, 3):
                xcorr_chunk(q, 0)
                xcorr_chunk(q, 1)

    nc.compile()
    return nc


def _host_prep(kernel, search, w_k, g_k, b_k, m_k, v_k, w_s, g_s, b_s, m_s, v_s):
    import ml_dtypes
    bf16 = ml_dtypes.bfloat16

    def fold(w, g, b, m, v):
        scale = g / np.sqrt(v + EPS)
        return (w * scale[:, None, None, None]).astype(np.float32), \
               (b - m * scale).astype(np.float32)

    wkf, bias_k = fold(w_k, g_k, b_k, m_k, v_k)
    wsf, bias_s = fold(w_s, g_s, b_s, m_s, v_s)

    def packT(w):  # [o, ci, 3, 3] -> [cb, ci, (t, ob, o)] bf16
        arr = w.reshape(2, 128, 2, 128, 9).transpose(2, 3, 4, 0, 1)
        return np.ascontiguousarray(arr, dtype=np.float32).astype(
            bf16).reshape(2, 128, 2304)

    wTk = packT(wkf)
    wTs = packT(wsf)

    M64 = np.zeros((128, 64), dtype=np.float32)
    for p in range(128):
        M64[p, p % 64] = 1.0
    M64REP = np.tile(M64, (1, 25)).astype(bf16)

    bk = np.ascontiguousarray(bias_k.reshape(2, 128, 1))
    bs = np.ascontiguousarray(bias_s.reshape(2, 128, 1))

    in_maps = []
    for core in range(N_CORES):
        kin = kernel[core * SPC:(core + 1) * SPC]
        sin = search[core * SPC:(core + 1) * SPC]

        Xk = np.zeros((2, 128, 9, 200), dtype=np.float32)
        for t in range(9):
            dy, dx = t // 3, t % 3
            p = kin[:, :, dy:dy + 5, dx:dx + 5].reshape(SPC, 2, 128, 25)
            Xk[:, :, t, :] = p.transpose(1, 2, 0, 3).reshape(2, 128, 200)
        Xk = Xk.astype(bf16).reshape(2, 128, 1800)

        Xs = np.zeros((SPC, 2, 128, 33, 34), dtype=np.float32)
        Xs[:, :, :, 1:32, 1:32] = sin.reshape(SPC, 2, 128, 31, 31)
        Xs = np.ascontiguousarray(
            Xs.transpose(0, 2, 1, 3, 4)).astype(bf16).reshape(
            SPC, 128, 2 * 33 * 34)

        in_maps.append({
            "wTs0": wTs[0], "wTs1": wTs[1],
            "wTk0": wTk[0], "wTk1": wTk[1],
            "xk0": Xk[0], "xk1": Xk[1],
            "xs": Xs, "bk": bk, "bs": bs, "m64rep": M64REP,
        })
    return in_maps


def kernel(kernel, search, w_k, g_k, b_k, m_k, v_k, w_s, g_s, b_s, m_s, v_s,
           _trace=False):
    global _cached_nc, last_results
    args = [np.ascontiguousarray(np.asarray(x, dtype=np.float32)) for x in
            (kernel, search, w_k, g_k, b_k, m_k, v_k, w_s, g_s, b_s, m_s, v_s)]
    if _cached_nc is None:
        _cached_nc = _build_program()
    nc = _cached_nc
    in_maps = _host_prep(*args)
    res = run_bass_kernel_spmd(nc, in_maps, core_ids=list(range(N_CORES)),
                               trace=_trace)
    last_results = res
    out = np.concatenate([res.results[i]["out"] for i in range(N_CORES)], axis=0)
    return np.ascontiguousarray(out.astype(np.float32))
